# revision 10
# baseline (speedup 1.0000x reference)
"""Pipeline v5c: fused depthwise+transpose via x-stationary banded matmuls.

Per core (4 batches):
  x host-prepped to [b, h, 114w, 128ci] bf16 (W zero-padded, ci zero-padded
  96->128 so LDWEIGHTS gets FWL, h on partitions)
  DW+TR fused: for each padded column w, stationary = x[:, w, :] [112h, 128ci];
    ONE matmul streams all valid taps: rhs = band3[:, t0:t0+k, :] where
    band3[:, t, :] = B_{v=2-t} (v-reversed band concat), writing k adjacent
    j-slots of a 2-bank PSUM tile [128ci, 8slot, 128] (j = w-v). Accumulation
    across w via PSUM has_written bits; one start=True per BANK (4 slots).
    Splits into 2 matmuls when the slot range crosses a bank boundary
    (168 MM/batch). -> q lands directly as [ci, i, j]: no PE transpose.
  q evac two-stage: scalar/vector copy PSUM f32 -> SBUF f32 (1cyc/elem),
    then GpSimd rearranging cast f32 -> bf16 into qtb [96ci, 112i, 112j].
  PW: lhsT = wpcT half [96ci, 96co] bf16, rhs = qtb[:, i0:i0+4, :] (N=448
    bf16), pairs into 2-bank PSUM tiles [96, 2, 512]; interleaved into the
    NEXT batch's DW stream (one pair per 4 w-iters) to keep the PE hot.
  y evac: plain f32 copy to SBUF, DMA f32 (no slow bf16 cast on this path).
"""

import numpy as np
import ml_dtypes

from concourse import bacc, mybir
from concourse import tile
from concourse.bass_utils import run_bass_kernel_spmd

F32 = mybir.dt.float32
BF16 = mybir.dt.bfloat16

B, C_IN, C_OUT, H, W = 32, 96, 192, 112, 112
N_CORES = 8
B_PER = B // N_CORES
WP = W + 2                      # 114 padded width
CIP = 128                       # ci padded for FWL
SLOTS = 8                       # j-columns per DW psum tile (2 banks)
N_BLOCKS = W // SLOTS           # 14 DW blocks per batch
N_PAIRS = 28                    # PW matmul pairs per batch

_NC = None
LAST_RESULTS = None


def _build():
    nc = bacc.Bacc("TRN2", target_bir_lowering=False, debug=False,
                   num_devices=N_CORES)

    x_d = nc.dram_tensor("x", [B_PER, H, WP, CIP], BF16, kind="ExternalInput")
    band3_d = nc.dram_tensor("band3", [H, 3, H], BF16, kind="ExternalInput")
    wpcT_d = nc.dram_tensor("wpcT", [C_IN, C_OUT], BF16, kind="ExternalInput")
    y_d = nc.dram_tensor("y", [B_PER, C_OUT, H, W], F32, kind="ExternalOutput")

    with tile.TileContext(nc) as tc:
        with (
            tc.tile_pool(name="consts", bufs=1) as consts,
            tc.tile_pool(name="xin", bufs=2) as xin,
            tc.tile_pool(name="qf", bufs=3) as qfp,
            tc.tile_pool(name="qtb", bufs=2) as qtbp,
            tc.tile_pool(name="ys", bufs=2) as ysp,
            tc.tile_pool(name="dwp", bufs=2, space="PSUM") as dwp,
            tc.tile_pool(name="ypp", bufs=2, space="PSUM") as ypp,
        ):
            band3_sb = consts.tile([H, 3, H], BF16)
            nc.sync.dma_start(band3_sb[:], band3_d[:])
            wpc_sb = consts.tile([C_IN, C_OUT], BF16)
            nc.sync.dma_start(wpc_sb[:], wpcT_d[:])

            ctrs = {"q": 0, "y": 0}

            def evac(kind, dst, src):
                if ctrs[kind] % 2 == 0:
                    nc.scalar.copy(dst, src)
                else:
                    nc.vector.tensor_copy(dst, src)
                ctrs[kind] += 1

            def load_x(b, xt):
                # 4 w-chunks so the first matmuls don't wait on the full slab
                for c in range(4):
                    w0 = [0, 32, 64, 96][c]
                    w1 = [32, 64, 96, WP][c]
                    nc.sync.dma_start(xt[:, w0:w1, :], x_d[b, :, w0:w1, :])

            xt_cur = xin.tile([H, WP, CIP], BF16, name="x0", tag="x")
            load_x(0, xt_cur)

            def pw_steps(bprev, qtb_prev):
                """Generator: one PW pair (2 matmuls + copy + maybe DMA)."""
                ys_cur = None
                for p in range(N_PAIRS):
                    blk, mt, h2 = p // 4, (p // 2) % 2, p % 2
                    if h2 == 0:
                        ys_cur = ysp.tile([C_IN, 16, W], F32,
                                          name=f"ys{mt}", tag="ys")
                    yt = ypp.tile([C_IN, 2, 512], F32, name="yp", tag="yp")
                    lhsT = wpc_sb[:, mt * 96:(mt + 1) * 96]
                    for k in range(2):
                        i0 = blk * 16 + h2 * 8 + k * 4
                        nc.tensor.matmul(
                            yt[:, k, 0:448], lhsT,
                            qtb_prev[:, i0:i0 + 4, :],
                            start=True, stop=True)
                    evac("y", ys_cur[:, h2 * 8:(h2 + 1) * 8, :],
                         yt[:, :, 0:448])
                    if h2 == 1:
                        nc.sync.dma_start(
                            y_d[bprev, mt * 96:(mt + 1) * 96,
                                blk * 16:(blk + 1) * 16, :],
                            ys_cur[:])
                    yield

            def evac_block(bd, ptile, qtb):
                qf = qfp.tile([C_IN, SLOTS, H], F32, name="qf", tag="qf")
                evac("q", qf[:], ptile[0:C_IN, :, 0:112])
                nc.gpsimd.tensor_copy(
                    qtb[:, :, SLOTS * bd:SLOTS * (bd + 1)]
                    .rearrange("c i j -> c j i"),
                    qf[:])

            pw_iter = None
            for b in range(B_PER):
                qtb = qtbp.tile([C_IN, H, W], BF16, name="qtb", tag="qtb")
                xt = xt_cur
                if b + 1 < B_PER:
                    xt_next = xin.tile([H, WP, CIP], BF16,
                                       name=f"x{b + 1}", tag="x")
                    load_x(b + 1, xt_next)
                    xt_cur = xt_next

                ptiles = {}          # block -> psum tile
                started = set()      # banks with start emitted
                for w in range(WP):
                    jlo, jhi = max(0, w - 2), min(W - 1, w)
                    lhsT = xt[:, w, :]
                    # group valid j-columns by psum BANK (4 j each)
                    groups = {}
                    for j in range(jlo, jhi + 1):
                        groups.setdefault(j // 4, []).append(j)
                    for bank in sorted(groups):
                        gj = groups[bank]
                        block = bank // 2
                        s0 = gj[0] % SLOTS
                        t0 = 2 - (w - gj[0])
                        k = len(gj)
                        if block not in ptiles:
                            ptiles[block] = dwp.tile([CIP, SLOTS, 128], F32,
                                                     name="dw", tag="dw")
                        start = bank not in started
                        started.add(bank)
                        stop = (gj[-1] == 4 * bank + 3 and w - gj[-1] == 2)
                        nc.tensor.matmul(
                            ptiles[block][:, s0:s0 + k, 0:112],
                            lhsT, band3_sb[:, t0:t0 + k, :],
                            start=start, stop=stop, skip_group_check=True)
                        if stop and bank % 2 == 1:
                            evac_block(block, ptiles.pop(block), qtb)
                    if w % 4 == 3 and pw_iter is not None:
                        next(pw_iter, None)
                assert not ptiles
                if pw_iter is not None:
                    for _ in pw_iter:
                        pass
                pw_iter = pw_steps(b, qtb)

            for _ in pw_iter:
                pass

    nc.compile()
    return nc


def _prep_inputs(x, w_pc, w_dc):
    x = np.asarray(x, dtype=np.float32)
    k3 = np.asarray(w_dc, dtype=np.float32).reshape(3, 3)
    Wm = np.asarray(w_pc, dtype=np.float32).reshape(C_OUT, C_IN)

    # [b, h, 114, 128]: transpose + W-pad + ci-pad
    xp = np.zeros((B, H, WP, CIP), dtype=np.float32)
    xp[:, :, 1:1 + W, 0:C_IN] = x.transpose(0, 2, 3, 1)

    # band3[h, t, i] = k3[h - i + 1, 2 - t]  (v-reversed band concat)
    band3 = np.zeros((H, 3, H), dtype=np.float32)
    hh, ii = np.meshgrid(np.arange(H), np.arange(H), indexing="ij")
    u = hh - ii + 1
    m = (u >= 0) & (u < 3)
    for t in range(3):
        bv = np.zeros((H, H), dtype=np.float32)
        bv[m] = k3[u[m], 2 - t]
        band3[:, t, :] = bv

    wpcT = np.ascontiguousarray(Wm.T)
    bf = ml_dtypes.bfloat16
    return (xp.astype(bf), band3.astype(bf), wpcT.astype(bf))


def kernel(x, w_pc, w_dc, _trace=False):
    global _NC, LAST_RESULTS
    if _NC is None:
        _NC = _build()

    xp, band3, wpcT = _prep_inputs(x, w_pc, w_dc)
    in_maps = [
        {"x": np.ascontiguousarray(xp[i * B_PER:(i + 1) * B_PER]),
         "band3": band3, "wpcT": wpcT}
        for i in range(N_CORES)
    ]
    res = run_bass_kernel_spmd(_NC, in_maps, list(range(N_CORES)),
                               trace=_trace)
    LAST_RESULTS = res
    y = np.concatenate([res.results[i]["y"] for i in range(N_CORES)], axis=0)
    return np.asarray(y, dtype=np.float32)


# revision 12
# speedup vs baseline: 1.1414x; 1.1414x over previous
"""Pipeline v5d: fused depthwise+transpose + flipped pointwise (bf16, FWL).

Per core (4 batches):
  x host-prepped to [b, h, 114w, 128ci] bf16 (W zero-padded, ci zero-padded
  96->128 so LDWEIGHTS gets FWL, h on partitions)
  DW+TR fused: for each padded column w, stationary = x[:, w, :] [112h, 128ci];
    ONE matmul streams all valid taps: rhs = band3[:, t0:t0+k, :] where
    band3[:, t, :] = B_{v=2-t}, writing k adjacent j-slots of a 2-bank PSUM
    tile [128ci, 8slot, 128] (j = w-v). Accumulation across w via PSUM
    has_written bits; one start=True per BANK. Splits in 2 when the slot run
    crosses a bank boundary (168 MM/batch). -> q lands as [ci, i, j].
  q evac: scalar/vector cast f32->bf16 -> qtb [96ci, (i j)] (2cyc/elem, the
    unavoidable cast).
  PW flipped: stationary = qtb 128-wide spatial window [96ci, 128] bf16
    (M=128 -> FWL), moving = wpcT [96ci, 192co] (N=192, ~80ns/MM), out =
    yT [128sp, 192co] f32; 98 windows/batch interleaved one per DW w-iter.
  y evac: plain f32 copy to SBUF staging, DMA f32, host transposes
    [b, sp, co] -> [b, co, h, w].
"""

import numpy as np
import ml_dtypes

from concourse import bacc, mybir
from concourse import tile
from concourse.bass_utils import run_bass_kernel_spmd

F32 = mybir.dt.float32
BF16 = mybir.dt.bfloat16

B, C_IN, C_OUT, H, W = 32, 96, 192, 112, 112
N_CORES = 8
B_PER = B // N_CORES
WP = W + 2                      # 114 padded width
CIP = 128                       # ci padded for FWL
SLOTS = 8                       # j-columns per DW psum tile (2 banks)
SP = H * W                      # 12544 spatial elements
N_WIN = SP // 128               # 98 PW windows per batch
WIN_PER_TILE = 4                # PW windows per psum tile (2 banks)
WIN_PER_YS = 8                  # PW windows per staged y DMA

_NC = None
LAST_RESULTS = None


def _build():
    nc = bacc.Bacc("TRN2", target_bir_lowering=False, debug=False,
                   num_devices=N_CORES)

    x_d = nc.dram_tensor("x", [B_PER, H, WP, CIP], BF16, kind="ExternalInput")
    band3_d = nc.dram_tensor("band3", [H, 3, H], BF16, kind="ExternalInput")
    wpcT_d = nc.dram_tensor("wpcT", [C_IN, C_OUT], BF16, kind="ExternalInput")
    y_d = nc.dram_tensor("y", [B_PER, N_WIN, 128, C_OUT], F32,
                         kind="ExternalOutput")

    with tile.TileContext(nc) as tc:
        with (
            tc.tile_pool(name="consts", bufs=1) as consts,
            tc.tile_pool(name="xin", bufs=2) as xin,
            tc.tile_pool(name="qtb", bufs=2) as qtbp,
            tc.tile_pool(name="ys", bufs=3) as ysp,
            tc.tile_pool(name="dwp", bufs=2, space="PSUM") as dwp,
            tc.tile_pool(name="ypp", bufs=2, space="PSUM") as ypp,
        ):
            band3_sb = consts.tile([H, 3, H], BF16)
            nc.sync.dma_start(band3_sb[:], band3_d[:])
            wpc_sb = consts.tile([C_IN, C_OUT], BF16)
            nc.sync.dma_start(wpc_sb[:], wpcT_d[:])

            ctrs = {"q": 0, "y": 0}

            def evac(kind, dst, src):
                if ctrs[kind] % 2 == 0:
                    nc.scalar.copy(dst, src)
                else:
                    nc.vector.tensor_copy(dst, src)
                ctrs[kind] += 1

            def load_x(b, xt):
                for c in range(4):
                    w0 = [0, 32, 64, 96][c]
                    w1 = [32, 64, 96, WP][c]
                    nc.sync.dma_start(xt[:, w0:w1, :], x_d[b, :, w0:w1, :])

            xt_cur = xin.tile([H, WP, CIP], BF16, name="x0", tag="x")
            load_x(0, xt_cur)

            def pw_steps(bprev, qtb_prev):
                """Generator: one flipped-PW matmul per step."""
                qflat = qtb_prev[:].rearrange("c i j -> c (i j)")
                ys_cur = None
                yt = None
                for wi in range(N_WIN):
                    if wi % WIN_PER_YS == 0:
                        ys_cur = ysp.tile([128, WIN_PER_YS, C_OUT], F32,
                                          name="ys", tag="ys")
                    if wi % WIN_PER_TILE == 0:
                        yt = ypp.tile([128, WIN_PER_TILE, 256], F32,
                                      name="yp", tag="yp")
                    k = wi % WIN_PER_TILE
                    nc.tensor.matmul(
                        yt[:, k, 0:C_OUT],
                        qflat[:, 128 * wi:128 * (wi + 1)],
                        wpc_sb[:],
                        start=True, stop=True)
                    if k == WIN_PER_TILE - 1 or wi == N_WIN - 1:
                        g = (wi % WIN_PER_YS) // WIN_PER_TILE
                        evac("y",
                             ys_cur[:, WIN_PER_TILE * g:
                                    WIN_PER_TILE * g + k + 1, :],
                             yt[:, 0:k + 1, 0:C_OUT])
                    if wi % WIN_PER_YS == WIN_PER_YS - 1 or wi == N_WIN - 1:
                        wi0 = (wi // WIN_PER_YS) * WIN_PER_YS
                        n = wi - wi0 + 1
                        nc.sync.dma_start(
                            y_d[bprev, wi0:wi0 + n, :, :]
                            .rearrange("w p c -> p w c"),
                            ys_cur[:, 0:n, :])
                    yield

            pw_iter = None
            for b in range(B_PER):
                qtb = qtbp.tile([C_IN, H, W], BF16, name="qtb", tag="qtb")
                xt = xt_cur
                if b + 1 < B_PER:
                    xt_next = xin.tile([H, WP, CIP], BF16,
                                       name=f"x{b + 1}", tag="x")
                    load_x(b + 1, xt_next)
                    xt_cur = xt_next

                ptiles = {}          # block -> psum tile
                started = set()      # banks with start emitted
                for w in range(WP):
                    jlo, jhi = max(0, w - 2), min(W - 1, w)
                    lhsT = xt[:, w, :]
                    groups = {}
                    for j in range(jlo, jhi + 1):
                        groups.setdefault(j // 4, []).append(j)
                    for bank in sorted(groups):
                        gj = groups[bank]
                        block = bank // 2
                        s0 = gj[0] % SLOTS
                        t0 = 2 - (w - gj[0])
                        k = len(gj)
                        if block not in ptiles:
                            ptiles[block] = dwp.tile([CIP, SLOTS, 128], F32,
                                                     name="dw", tag="dw")
                        start = bank not in started
                        started.add(bank)
                        stop = (gj[-1] == 4 * bank + 3 and w - gj[-1] == 2)
                        nc.tensor.matmul(
                            ptiles[block][:, s0:s0 + k, 0:112],
                            lhsT, band3_sb[:, t0:t0 + k, :],
                            start=start, stop=stop, skip_group_check=True)
                        if stop and bank % 2 == 1:
                            pt = ptiles.pop(block)
                            evac("q",
                                 qtb[:, :, SLOTS * block:SLOTS * (block + 1)]
                                 .rearrange("c i j -> c j i"),
                                 pt[0:C_IN, :, 0:112])
                    if 8 <= w < 106 and pw_iter is not None:
                        next(pw_iter, None)
                assert not ptiles
                if pw_iter is not None:
                    for _ in pw_iter:
                        pass
                pw_iter = pw_steps(b, qtb)

            for _ in pw_iter:
                pass

    nc.compile()
    return nc


def _prep_inputs(x, w_pc, w_dc):
    x = np.asarray(x, dtype=np.float32)
    k3 = np.asarray(w_dc, dtype=np.float32).reshape(3, 3)
    Wm = np.asarray(w_pc, dtype=np.float32).reshape(C_OUT, C_IN)

    xp = np.zeros((B, H, WP, CIP), dtype=np.float32)
    xp[:, :, 1:1 + W, 0:C_IN] = x.transpose(0, 2, 3, 1)

    band3 = np.zeros((H, 3, H), dtype=np.float32)
    hh, ii = np.meshgrid(np.arange(H), np.arange(H), indexing="ij")
    u = hh - ii + 1
    m = (u >= 0) & (u < 3)
    for t in range(3):
        bv = np.zeros((H, H), dtype=np.float32)
        bv[m] = k3[u[m], 2 - t]
        band3[:, t, :] = bv

    wpcT = np.ascontiguousarray(Wm.T)
    bf = ml_dtypes.bfloat16
    return (xp.astype(bf), band3.astype(bf), wpcT.astype(bf))


def kernel(x, w_pc, w_dc, _trace=False):
    global _NC, LAST_RESULTS
    if _NC is None:
        _NC = _build()

    xp, band3, wpcT = _prep_inputs(x, w_pc, w_dc)
    in_maps = [
        {"x": np.ascontiguousarray(xp[i * B_PER:(i + 1) * B_PER]),
         "band3": band3, "wpcT": wpcT}
        for i in range(N_CORES)
    ]
    res = run_bass_kernel_spmd(_NC, in_maps, list(range(N_CORES)),
                               trace=_trace)
    LAST_RESULTS = res
    # y arrives [B_PER, 98, 128, 192] = [b, (h w), co] -> [b, co, h, w]
    y = np.concatenate([np.asarray(res.results[i]["y"], dtype=np.float32)
                        for i in range(N_CORES)], axis=0)
    y = y.reshape(B, H, W, C_OUT).transpose(0, 3, 1, 2)
    return np.ascontiguousarray(y)


# revision 18
# speedup vs baseline: 1.9625x; 1.7195x over previous
"""Pipeline v5d: fused depthwise+transpose + flipped pointwise (bf16, FWL).

Per core (4 batches):
  x host-prepped to [b, h, 114w, 128ci] bf16 (W zero-padded, ci zero-padded
  96->128 so LDWEIGHTS gets FWL, h on partitions)
  DW+TR fused: for each padded column w, stationary = x[:, w, :] [112h, 128ci];
    ONE matmul streams all valid taps: rhs = band3[:, t0:t0+k, :] where
    band3[:, t, :] = B_{v=2-t}, writing k adjacent j-slots of a 2-bank PSUM
    tile [128ci, 8slot, 128] (j = w-v). Accumulation across w via PSUM
    has_written bits; one start=True per BANK. Splits in 2 when the slot run
    crosses a bank boundary (168 MM/batch). -> q lands as [ci, i, j].
  q evac: scalar/vector cast f32->bf16 -> qtb [96ci, (i j)] (2cyc/elem, the
    unavoidable cast).
  PW flipped: stationary = qtb 128-wide spatial window [96ci, 128] bf16
    (M=128 -> FWL), moving = wpcT [96ci, 192co] (N=192, ~80ns/MM), out =
    yT [128sp, 192co] f32; 98 windows/batch interleaved one per DW w-iter.
  y evac: plain f32 copy to SBUF staging, DMA f32, host transposes
    [b, sp, co] -> [b, co, h, w].
"""

import numpy as np
import ml_dtypes

from concourse import bacc, mybir
from concourse import tile
from concourse.bass_utils import run_bass_kernel_spmd

F32 = mybir.dt.float32
BF16 = mybir.dt.bfloat16

B, C_IN, C_OUT, H, W = 32, 96, 192, 112, 112
N_CORES = 8
B_PER = B // N_CORES
WP = W + 2                      # 114 padded width
CIP = 128                       # ci padded for FWL
SLOTS = 8                       # j-columns per DW psum tile (2 banks)
SP = H * W                      # 12544 spatial elements
N_WIN = SP // 128               # 98 PW windows per batch
WIN_PER_TILE = 4                # PW windows per psum tile (2 banks)
WIN_PER_YS = 8                  # PW windows per staged y DMA

_NC = None
LAST_RESULTS = None


def _build():
    nc = bacc.Bacc("TRN2", target_bir_lowering=False, debug=False,
                   num_devices=N_CORES)

    x_d = nc.dram_tensor("x", [B_PER, H, WP, CIP], BF16, kind="ExternalInput")
    band3_d = nc.dram_tensor("band3", [H, 3, H], BF16, kind="ExternalInput")
    wpcT_d = nc.dram_tensor("wpcT", [C_IN, C_OUT], BF16, kind="ExternalInput")
    y_d = nc.dram_tensor("y", [B_PER, 128, N_WIN, C_OUT], F32,
                         kind="ExternalOutput")

    with tile.TileContext(nc) as tc:
        with (
            tc.tile_pool(name="consts", bufs=1) as consts,
            tc.tile_pool(name="xin", bufs=2) as xin,
            tc.tile_pool(name="qtb", bufs=2) as qtbp,
            tc.tile_pool(name="ys", bufs=3) as ysp,
            tc.tile_pool(name="dwp", bufs=2, space="PSUM") as dwp,
            tc.tile_pool(name="ypp", bufs=2, space="PSUM") as ypp,
        ):
            band3_sb = consts.tile([H, 3, H], BF16)
            nc.sync.dma_start(band3_sb[:], band3_d[:])
            wpc_sb = consts.tile([C_IN, C_OUT], BF16)
            nc.sync.dma_start(wpc_sb[:], wpcT_d[:])

            ctrs = {"q": 0, "y": 0}

            def evac(kind, dst, src):
                if ctrs[kind] % 2 == 0:
                    nc.scalar.copy(dst, src)
                else:
                    nc.vector.tensor_copy(dst, src)
                ctrs[kind] += 1

            def load_x(b, xt):
                for c in range(4):
                    w0 = [0, 32, 64, 96][c]
                    w1 = [32, 64, 96, WP][c]
                    nc.sync.dma_start(xt[:, w0:w1, :], x_d[b, :, w0:w1, :])

            xt_cur = xin.tile([H, WP, CIP], BF16, name="x0", tag="x")
            load_x(0, xt_cur)

            def pw_steps(bprev, qtb_prev):
                """Generator: one flipped-PW matmul per step."""
                qflat = qtb_prev[:].rearrange("c j i -> c (j i)")
                ys_cur = None
                yt = None
                for wi in range(N_WIN):
                    if wi % WIN_PER_YS == 0:
                        ys_cur = ysp.tile([128, WIN_PER_YS, C_OUT], F32,
                                          name="ys", tag="ys")
                    if wi % WIN_PER_TILE == 0:
                        yt = ypp.tile([128, WIN_PER_TILE, 256], F32,
                                      name="yp", tag="yp")
                    k = wi % WIN_PER_TILE
                    nc.tensor.matmul(
                        yt[:, k, 0:C_OUT],
                        qflat[:, 128 * wi:128 * (wi + 1)],
                        wpc_sb[:],
                        start=True, stop=True)
                    if k == WIN_PER_TILE - 1 or wi == N_WIN - 1:
                        g = (wi % WIN_PER_YS) // WIN_PER_TILE
                        evac("y",
                             ys_cur[:, WIN_PER_TILE * g:
                                    WIN_PER_TILE * g + k + 1, :],
                             yt[:, 0:k + 1, 0:C_OUT])
                    if wi % WIN_PER_YS == WIN_PER_YS - 1 or wi == N_WIN - 1:
                        wi0 = (wi // WIN_PER_YS) * WIN_PER_YS
                        n = wi - wi0 + 1
                        nc.sync.dma_start(
                            y_d[bprev, :, wi0:wi0 + n, :],
                            ys_cur[:, 0:n, :])
                    yield

            pw_iter = None
            for b in range(B_PER):
                # j-major: [ci, j, i] so the DW evac dst is contiguous
                qtb = qtbp.tile([C_IN, W, H], BF16, name="qtb", tag="qtb")
                xt = xt_cur
                if b + 1 < B_PER:
                    xt_next = xin.tile([H, WP, CIP], BF16,
                                       name=f"x{b + 1}", tag="x")
                    load_x(b + 1, xt_next)
                    xt_cur = xt_next

                ptiles = {}          # block -> psum tile
                started = set()      # banks with start emitted
                for w in range(WP):
                    jlo, jhi = max(0, w - 2), min(W - 1, w)
                    lhsT = xt[:, w, :]
                    groups = {}
                    for j in range(jlo, jhi + 1):
                        groups.setdefault(j // 4, []).append(j)
                    for bank in sorted(groups):
                        gj = groups[bank]
                        block = bank // 2
                        s0 = gj[0] % SLOTS
                        t0 = 2 - (w - gj[0])
                        k = len(gj)
                        if block not in ptiles:
                            ptiles[block] = dwp.tile([CIP, SLOTS, 128], F32,
                                                     name="dw", tag="dw")
                        start = bank not in started
                        started.add(bank)
                        stop = (gj[-1] == 4 * bank + 3 and w - gj[-1] == 2)
                        nc.tensor.matmul(
                            ptiles[block][:, s0:s0 + k, 0:112],
                            lhsT, band3_sb[:, t0:t0 + k, :],
                            start=start, stop=stop, skip_group_check=True)
                        if stop and bank % 2 == 1:
                            pt = ptiles.pop(block)
                            evac("q",
                                 qtb[:, SLOTS * block:SLOTS * (block + 1), :],
                                 pt[0:C_IN, :, 0:112])
                    if 8 <= w < 106 and pw_iter is not None:
                        next(pw_iter, None)
                assert not ptiles
                if pw_iter is not None:
                    for _ in pw_iter:
                        pass
                pw_iter = pw_steps(b, qtb)

            for _ in pw_iter:
                pass

    nc.compile()
    return nc


def _prep_inputs(x, w_pc, w_dc):
    x = np.asarray(x, dtype=np.float32)
    k3 = np.asarray(w_dc, dtype=np.float32).reshape(3, 3)
    Wm = np.asarray(w_pc, dtype=np.float32).reshape(C_OUT, C_IN)

    xp = np.zeros((B, H, WP, CIP), dtype=np.float32)
    xp[:, :, 1:1 + W, 0:C_IN] = x.transpose(0, 2, 3, 1)

    band3 = np.zeros((H, 3, H), dtype=np.float32)
    hh, ii = np.meshgrid(np.arange(H), np.arange(H), indexing="ij")
    u = hh - ii + 1
    m = (u >= 0) & (u < 3)
    for t in range(3):
        bv = np.zeros((H, H), dtype=np.float32)
        bv[m] = k3[u[m], 2 - t]
        band3[:, t, :] = bv

    wpcT = np.ascontiguousarray(Wm.T)
    bf = ml_dtypes.bfloat16
    return (xp.astype(bf), band3.astype(bf), wpcT.astype(bf))


def kernel(x, w_pc, w_dc, _trace=False):
    global _NC, LAST_RESULTS
    if _NC is None:
        _NC = _build()

    xp, band3, wpcT = _prep_inputs(x, w_pc, w_dc)
    in_maps = [
        {"x": np.ascontiguousarray(xp[i * B_PER:(i + 1) * B_PER]),
         "band3": band3, "wpcT": wpcT}
        for i in range(N_CORES)
    ]
    res = run_bass_kernel_spmd(_NC, in_maps, list(range(N_CORES)),
                               trace=_trace)
    LAST_RESULTS = res
    # y arrives [B_PER, 128p, 98wi, 192co]; spatial (j i) flat = wi*128 + p
    y = np.concatenate([np.asarray(res.results[i]["y"], dtype=np.float32)
                        for i in range(N_CORES)], axis=0)
    y = y.transpose(0, 2, 1, 3).reshape(B, W, H, C_OUT)
    return np.ascontiguousarray(y.transpose(0, 3, 2, 1))
